# revision 3
# baseline (speedup 1.0000x reference)
"""Trainium2 Bass kernel for the CellLoss problem.

loss = mean_i [ 1/(x[i, l_i] + 0.1) + sum_j x[i,j] * (x[i,j] > x[i, l_i]) ]
with x: [131072, 256] f32, l: [131072] int labels in [0, 256).

Pure data parallel across 8 NeuronCores (16384 rows each). Per core,
partition p owns rows [p*128, (p+1)*128) of the shard; tile t is the
[128, 256] block of row p*128+t per partition.

Per tile, two fused ops (scalar_tensor_tensor = (in0 op0 scalar) op1 in1
with a per-row sum accumulator):
  gather  g[p] = sum_j (iota==l_p)*x  (exact: one-hot extract)
  margin  m[p] = sum_j (x > g_p)*x
Tiles are round-robined over engines by PATTERN: "D" = both on DVE,
"G" = both on GPSIMD, "A" = gather on DVE + margin on the scalar engine
as relu-accum + sign-accum (margin = relu_sum + g*(sign_sum+255)/2).
Tail: inv = 1/(g+0.1); totals; partition sum via ones-matmul on PE; one
f32 partial per core; host sums 8 partials / B.

This walrus build allows only ONE sync wait per instruction; Tile can
emit several (e.g. the tail drain). _split_multi_waits() rewrites extra
waits onto single-wait Drain carriers after scheduling.
"""

import numpy as np
from contextlib import ExitStack

import concourse.bass as bass
import concourse.mybir as mybir
import concourse.tile as tile
from concourse.bass_utils import run_bass_kernel_spmd

F32 = mybir.dt.float32

B, C = 131072, 256
N_CORES = 8
B_LOCAL = B // N_CORES          # 16384
P = 128
N_TILES = B_LOCAL // P          # 128
TILES_PER_DMA = 16              # [128, 4096] f32 = 2 MiB per DMA
N_CHUNKS = N_TILES // TILES_PER_DMA

# engine per tile, cyclic over the 128 tiles ("D" / "G" / "A")
PATTERN = ["D"]

_NC_CACHE = {}
LAST_RESULTS = None
SPLIT_WAITS = True   # off for CoreSim (its event loop rejects bare Drains)
TRACE = False
TRACE_KW = {}


def _split_multi_waits(nc):
    """Walrus here accepts one sync-wait per instruction; hoist extras
    onto single-wait Drain carriers just before the instruction."""
    for f in nc.m.functions:
        for blk in f.blocks:
            insts = list(blk.instructions)
            out = []
            changed = False
            for inst in insts:
                si = inst.sync_info
                if si is not None and si.on_wait is not None and len(si.on_wait) > 1:
                    waits = list(si.on_wait)
                    for w in waits[:-1]:
                        d = mybir.InstDrain(
                            name=nc.get_next_instruction_name(),
                            ins=[], outs=[], bass_is_fusable=False)
                        d.engine = inst.engine
                        d.sync_info = mybir.SyncInfo(on_wait=[w], on_update=[])
                        out.append(d)
                    inst.sync_info = mybir.SyncInfo(
                        on_wait=[waits[-1]], on_update=list(si.on_update or []))
                    changed = True
                out.append(inst)
            if changed:
                blk.instructions = out


def _assignment():
    return [PATTERN[t % len(PATTERN)] for t in range(N_TILES)]


def build_nc():
    key = (tuple(_assignment()), SPLIT_WAITS)
    if key in _NC_CACHE:
        return _NC_CACHE[key]

    assign = _assignment()
    a_tiles = [t for t, a in enumerate(assign) if a == "A"]
    acol = {t: i for i, t in enumerate(a_tiles)}
    n_a = len(a_tiles)

    nc = bass.Bass()
    x = nc.declare_dram_parameter("x", [B_LOCAL, C], F32, isOutput=False)
    lbl = nc.declare_dram_parameter("lbl", [P, N_TILES], F32, isOutput=False)
    out = nc.declare_dram_parameter("out", [1, 1], F32, isOutput=True)

    xv = x.rearrange("(p t) c -> p (t c)", p=P, t=N_TILES)

    with tile.TileContext(nc) as tc, ExitStack() as ctx:
        singles = ctx.enter_context(tc.tile_pool(name="singles", bufs=1))
        xpool = ctx.enter_context(tc.tile_pool(name="x", bufs=3))
        scr = ctx.enter_context(tc.tile_pool(name="scr", bufs=3))
        psum = ctx.enter_context(tc.tile_pool(name="psum", bufs=1, space="PSUM"))

        lbl_sb = singles.tile([P, N_TILES], F32)
        nc.sync.dma_start(lbl_sb[:], lbl[:])

        iota_i = singles.tile([P, C], mybir.dt.int32)
        nc.gpsimd.iota(iota_i[:], pattern=[[1, C]], base=0, channel_multiplier=0)
        iota_f = singles.tile([P, C], F32)
        nc.vector.tensor_copy(iota_f[:], iota_i[:])

        G = singles.tile([P, N_TILES], F32)     # gathered g, all tiles
        M = singles.tile([P, N_TILES], F32)     # margin, D/G tiles ("A" cols 0)
        if n_a:
            nc.vector.memset(M[:], 0.0)
            RS = singles.tile([P, n_a], F32)    # sum relu(x-g)
            SS = singles.tile([P, n_a], F32)    # sum sign(x-g)
            NGC = singles.tile([P, n_a], F32)   # -g for ACT bias

        for chunk in range(N_CHUNKS):
            xw = xpool.tile([P, TILES_PER_DMA * C], F32)
            nc.sync.dma_start(
                xw[:],
                xv[:, chunk * TILES_PER_DMA * C:(chunk + 1) * TILES_PER_DMA * C])
            for k in range(TILES_PER_DMA):
                t = chunk * TILES_PER_DMA + k
                xb = xw[:, k * C:(k + 1) * C]
                a = assign[t]
                lc = lbl_sb[:, t:t + 1]
                gc = G[:, t:t + 1]
                if a == "G":
                    ge = scr.tile([P, C], F32, tag="ge")
                    nc.gpsimd.scalar_tensor_tensor(
                        out=ge[:], in0=iota_f[:], scalar=lc, in1=xb,
                        op0=mybir.AluOpType.is_equal, op1=mybir.AluOpType.mult,
                        accum_out=gc)
                    gm = scr.tile([P, C], F32, tag="gm")
                    nc.gpsimd.scalar_tensor_tensor(
                        out=gm[:], in0=xb, scalar=gc, in1=xb,
                        op0=mybir.AluOpType.is_gt, op1=mybir.AluOpType.mult,
                        accum_out=M[:, t:t + 1])
                else:
                    sel = scr.tile([P, C], F32, tag="sel")
                    nc.vector.scalar_tensor_tensor(
                        out=sel[:], in0=iota_f[:], scalar=lc, in1=xb,
                        op0=mybir.AluOpType.is_equal, op1=mybir.AluOpType.mult,
                        accum_out=gc)
                    if a == "D":
                        mp = scr.tile([P, C], F32, tag="mp")
                        nc.vector.scalar_tensor_tensor(
                            out=mp[:], in0=xb, scalar=gc, in1=xb,
                            op0=mybir.AluOpType.is_gt, op1=mybir.AluOpType.mult,
                            accum_out=M[:, t:t + 1])
                    else:  # "A"
                        j = acol[t]
                        nc.vector.tensor_scalar_mul(NGC[:, j:j + 1], gc, -1.0)
                        ar = scr.tile([P, C], F32, tag="ar")
                        nc.scalar.activation(
                            ar[:], xb, mybir.ActivationFunctionType.Relu,
                            bias=NGC[:, j:j + 1], scale=1.0,
                            accum_out=RS[:, j:j + 1])
                        asg = scr.tile([P, C], F32, tag="asg")
                        nc.scalar.activation(
                            asg[:], xb, mybir.ActivationFunctionType.Sign,
                            bias=NGC[:, j:j + 1], scale=1.0,
                            accum_out=SS[:, j:j + 1])

        # ---- tail ------------------------------------------------------
        tmp = scr.tile([P, N_TILES], F32, tag="tail")
        nc.vector.tensor_scalar_add(tmp[:], G[:], 0.1)
        inv = scr.tile([P, N_TILES], F32, tag="tail2")
        nc.vector.reciprocal(inv[:], tmp[:])
        tot = scr.tile([P, N_TILES], F32, tag="tail3")
        nc.vector.tensor_tensor(out=tot[:], in0=inv[:], in1=M[:],
                                op=mybir.AluOpType.add)
        rows = singles.tile([P, 1], F32)
        if n_a:
            # margin_a = RS + g_a * (SS + 255)/2, g_a = strided view of G
            k = len(PATTERN)
            a0 = PATTERN.index("A")
            assert a_tiles == list(range(a0, N_TILES, k)), \
                "A tiles must form a uniform stride for the strided G view"
            g_a = G.rearrange("p (t u) -> p t u", u=k)[:, :, a0]
            cnt = scr.tile([P, n_a], F32, tag="tailA")
            nc.vector.tensor_scalar(out=cnt[:], in0=SS[:], scalar1=255.0,
                                    scalar2=0.5, op0=mybir.AluOpType.add,
                                    op1=mybir.AluOpType.mult)
            gw = scr.tile([P, n_a], F32, tag="tailB")
            nc.vector.tensor_tensor(out=gw[:], in0=cnt[:], in1=g_a,
                                    op=mybir.AluOpType.mult)
            ta = scr.tile([P, n_a], F32, tag="tailC")
            nc.vector.tensor_tensor(out=ta[:], in0=gw[:], in1=RS[:],
                                    op=mybir.AluOpType.add)
            rows_a = singles.tile([P, 1], F32)
            nc.vector.tensor_reduce(rows_a[:], ta[:],
                                    axis=mybir.AxisListType.X,
                                    op=mybir.AluOpType.add)
            rows_dg = singles.tile([P, 1], F32)
            nc.vector.tensor_reduce(rows_dg[:], tot[:],
                                    axis=mybir.AxisListType.X,
                                    op=mybir.AluOpType.add)
            nc.vector.tensor_tensor(out=rows[:], in0=rows_dg[:],
                                    in1=rows_a[:], op=mybir.AluOpType.add)
        else:
            nc.vector.tensor_reduce(rows[:], tot[:],
                                    axis=mybir.AxisListType.X,
                                    op=mybir.AluOpType.add)

        ones = singles.tile([P, 1], F32)
        nc.vector.memset(ones[:], 1.0)
        ps = psum.tile([P, 2], F32)
        nc.tensor.matmul(ps[:1, :1], ones[:], rows[:])
        res = singles.tile([1, 1], F32)
        nc.vector.tensor_copy(res[:], ps[:1, :1])
        nc.sync.dma_start(out[:], res[:])

    if SPLIT_WAITS:
        _split_multi_waits(nc)
    _NC_CACHE[key] = nc
    return nc


def _prep_inputs(rna_cell_out, rna_cell_label):
    x = np.ascontiguousarray(np.asarray(rna_cell_out, dtype=np.float32))
    l = np.asarray(rna_cell_label).astype(np.int64)
    assert x.shape == (B, C) and l.shape == (B,)
    in_maps = []
    for i in range(N_CORES):
        xs = x[i * B_LOCAL:(i + 1) * B_LOCAL]
        ls = l[i * B_LOCAL:(i + 1) * B_LOCAL]
        lbl = ls.reshape(P, N_TILES).astype(np.float32)
        in_maps.append({"x": xs, "lbl": np.ascontiguousarray(lbl)})
    return in_maps


def kernel(rna_cell_out, rna_cell_label):
    global LAST_RESULTS
    nc = build_nc()
    in_maps = _prep_inputs(rna_cell_out, rna_cell_label)
    res = run_bass_kernel_spmd(nc, in_maps, list(range(N_CORES)),
                               trace=TRACE, **TRACE_KW)
    LAST_RESULTS = res
    parts = [float(res.results[i]["out"][0, 0]) for i in range(N_CORES)]
    loss = np.float32(np.sum(np.array(parts, dtype=np.float64)) / B)
    return np.array([loss], dtype=np.float32)
